# revision 22
# baseline (speedup 1.0000x reference)
"""PointHop octant-binning kernel for TRN2 (8 NeuronCores, B-sharded).

v5 design: [k,g]-transposed layout + PE (TensorEngine) reduction.

Per stripe of 1024 groups (32 stripes/core):
1. DMA gx -> V f32 [128 groups, (3c, 8t, 64k)]
2. ACT casts f32->fp16 (Vh)
3. PE transposes 12 [128,128] blocks: (t-pair, k) free -> partitions,
   giving Vk [(h,k)=128, (c, tau, 128 p-cols)] in PSUM; ACT copies to SBUF
4. DVE builds 34 fp16 arrays in SBUF: masks via is_gt (4x), products via
   TT mult (2x), relu lattice via tensor_scalar_max (4x), squares (2x)
5. PE: 35 accumulating matmuls with Moebius-weighted {0,+-1} stationaries:
   fuses the K-reduction AND the octant inclusion-exclusion butterfly into
   one PSUM accumulation chain MBS [76, 512] f32
6. ACT copy -> PE transposes back -> FIN [128 pair-cols, (tau,m',h)]
7. Small DVE/ACT epilogue (count clamp, reciprocal, means, std) + DMA out

Array index m = c*8 + T (T = mask-subset bitmask: 4=mx, 2=my, 1=mz).
Output rows m' = u*3+c (24 octant sums), 24+u (8 counts), 32+c (sum sq),
35+c (plain sums).
"""

import os
from contextlib import ExitStack

import numpy as np

if "axon" not in os.environ.get("JAX_PLATFORMS", "axon"):
    os.environ.pop("JAX_PLATFORMS", None)

import concourse.bass as bass
import concourse.bacc as bacc
import concourse.tile as tile
from concourse import mybir
from concourse.bass_utils import run_bass_kernel_spmd

B, C, N, K = 32, 3, 8192, 64
NCORES = 8
BL = B // NCORES          # 4 batches per core
PART = 128
SLAB = 1024               # groups per stripe
NSTRIPE = BL * N // SLAB  # 32
FOUT = 30
J = 512                   # pair-columns per stripe
NARR = 34                 # moving arrays per stripe (ones folded to epilogue)
NOUT = 38                 # output feature rows per group-half
QCOL = NOUT * 2           # 76 stationary columns

AL = mybir.AluOpType
AF = mybir.ActivationFunctionType
F32 = mybir.dt.float32
FP16 = mybir.dt.float16


def _moebius_weights() -> np.ndarray:
    """W[a, m'] over 35 arrays x 38 outputs."""
    W = np.zeros((NARR, NOUT), dtype=np.float32)

    def pc(x):
        return bin(x).count("1")

    def moeb(u, T):
        # octant sum: oct[u] = sum_{T >= u} (-1)^{|T|-|u|} S_T
        if (T & u) == u:
            return float((-1) ** (pc(T) - pc(u)))
        return 0.0

    # value arrays a = c*8 + T
    for c in range(3):
        for T in range(8):
            a = c * 8 + T
            for u in range(8):
                W[a, u * 3 + c] = moeb(u, T)
            if T == 0:
                W[a, 35 + c] = 1.0  # plain sum passthrough
    # count arrays: a = 24..30 for T in (1,2,4,3,5,6,7); the T=0 (ones)
    # contribution is the constant 64 added to u=0 in the epilogue
    cnt_T = [1, 2, 4, 3, 5, 6, 7]
    for i, T in enumerate(cnt_T):
        for u in range(8):
            W[24 + i, 24 + u] = moeb(u, T)
    # squares a = 31+c
    for c in range(3):
        W[31 + c, 32 + c] = 1.0
    return W


def _stationaries() -> np.ndarray:
    """ST[p, a*76 + m'*2 + h]: Moebius weight, gated on h-block of p."""
    W = _moebius_weights()
    ST = np.zeros((PART, NARR * QCOL), dtype=np.float16)
    for a in range(NARR):
        for mp in range(NOUT):
            w = W[a, mp]
            if w == 0.0:
                continue
            for h in range(2):
                ST[h * 64:(h + 1) * 64, a * QCOL + mp * 2 + h] = w
    return ST


def _build_kernel(nc: bass.Bass):
    gx = nc.dram_tensor("gx", [BL, C, N, K], F32, kind="ExternalInput")
    nx = nc.dram_tensor("nx", [BL, N, C], F32, kind="ExternalInput")
    mst = nc.dram_tensor("mst", [PART, NARR * QCOL], FP16,
                         kind="ExternalInput")
    ident = nc.dram_tensor("ident", [PART, PART], FP16, kind="ExternalInput")
    out = nc.dram_tensor("out", [BL, N, FOUT], F32, kind="ExternalOutput")

    ts = None

    with tile.TileContext(nc) as tc, ExitStack() as ctx:
        spool = ctx.enter_context(tc.tile_pool(name="s", bufs=1))
        vpool = ctx.enter_context(tc.tile_pool(name="v", bufs=3))
        epool = ctx.enter_context(tc.tile_pool(name="e", bufs=3))
        pmb = ctx.enter_context(tc.tile_pool(name="pmb", bufs=3,
                                             space="PSUM"))
        pfn = ctx.enter_context(tc.tile_pool(name="pfn", bufs=3,
                                             space="PSUM"))

        ts = nc.vector.tensor_scalar
        tt = nc.vector.tensor_tensor
        act = nc.scalar.activation

        # static tiles
        ST = spool.tile([PART, NARR * QCOL], FP16, name="ST")
        ID = spool.tile([PART, PART], FP16, name="ID")
        nc.sync.dma_start(out=ST[:], in_=mst[:, :])
        nc.sync.dma_start(out=ID[:], in_=ident[:, :])

        for s in range(NSTRIPE):
            b, blk = divmod(s, N // SLAB)
            n0 = blk * SLAB

            V = vpool.tile([PART, C * 512], F32, name="V")
            nc.sync.dma_start(
                out=V[:].rearrange("p (c t k) -> p c t k", c=C, t=8),
                in_=gx[b, :, n0:n0 + SLAB, :].rearrange(
                    "c (p t) k -> p c t k", p=PART, t=8))

            VH = vpool.tile([PART, C * 512], FP16, name="VH")
            act(VH[:], V[:], AF.Copy)

            # ---- transpose to [ (h,k), (c,tau,p) ] via DMA xbar ----
            VK = vpool.tile([PART, C * 512], FP16, name="VK")
            nc.sync.dma_start_transpose(
                out=VK[:].rearrange("p (b f) -> p b f", b=12), in_=VH[:])

            # ---- build arrays ----
            MK = vpool.tile([PART, C * J], FP16, name="MK")   # mx,my,mz
            MEGA = vpool.tile([PART, 24 * J], FP16, name="MEGA")
            CNT = vpool.tile([PART, 4 * J], FP16, name="CNT")
            SQ = vpool.tile([PART, C * J], FP16, name="SQ")

            vkc = VK[:].rearrange("p (c j) -> p c j", c=C)
            mkc = MK[:].rearrange("p (c j) -> p c j", c=C)
            mg = MEGA[:].rearrange("p (c t j) -> p c t j", c=C, t=8)

            # masks mx,my,mz (4x)
            ts(MK[:], VK[:], 0.0, None, AL.is_gt)

            # copy values into T=0 slots (4x)
            nc.vector.tensor_copy(mg[:, :, 0, :], vkc)

            def bcast(ap, n):
                return ap[:, None, :].broadcast_to([PART, n, J])

            # products (2x): dst m=c*8+T
            # P1: m1 (x*mz), m9 (y*mz)
            tt(mg[:, 0:2, 1, :], vkc[:, 0:2, :], bcast(mkc[:, 2, :], 2),
               AL.mult)
            # P2: m2 (x*my), m18 (z*my)
            mg24 = MEGA[:].rearrange("p (a j) -> p a j", a=24)
            tt(mg24[:, 2:24:16, :], vkc[:, 0:3:2, :],
               bcast(mkc[:, 1, :], 2), AL.mult)
            # P3: m3 = my * m1
            tt(mg[:, 0, 3, :], mg[:, 0, 1, :], mkc[:, 1, :], AL.mult)
            # P4: m12 (y*mx), m20 (z*mx)
            tt(mg[:, 1:3, 4, :], vkc[:, 1:3, :], bcast(mkc[:, 0, :], 2),
               AL.mult)
            # P5: m13 = mx * m9
            tt(mg[:, 1, 5, :], mg[:, 1, 1, :], mkc[:, 0, :], AL.mult)
            # P6: m22 = mx * m18
            tt(mg[:, 2, 6, :], mg[:, 2, 2, :], mkc[:, 0, :], AL.mult)

            # relus (4x): add own-coordinate mask
            nc.vector.tensor_scalar_max(mg[:, 0, 4:8, :], mg[:, 0, 0:4, :],
                                        0.0)
            my4 = MEGA[:].rearrange("p (c e d j) -> p c e d j",
                                    c=3, e=2, d=2)
            nc.vector.tensor_scalar_max(my4[:, 1, :, 1, :], my4[:, 1, :, 0, :],
                                        0.0)
            mz2 = MEGA[:].rearrange("p (c t d j) -> p c t d j",
                                    c=3, t=4, d=2)
            nc.vector.tensor_scalar_max(mz2[:, 2, :, 1, :], mz2[:, 2, :, 0, :],
                                        0.0)

            # count composite masks (4x): T=3 from m3? no:
            # CNT order T = 3,5,6,7 from is_gt of m9(z... see below
            # myz = 1[y*mz>0] (m9); mxz = 1[x*mz>0] (m1);
            # mxy = 1[x*my>0] (m2); mxyz = 1[x*my*mz>0] (m3)
            cn = CNT[:].rearrange("p (a j) -> p a j", a=4)
            ts(cn[:, 0, :], mg[:, 1, 1, :], 0.0, None, AL.is_gt)   # myz
            ts(cn[:, 1:4, :], mg[:, 0, 1:4, :], 0.0, None, AL.is_gt)

            # squares (2x)
            tt(SQ[:], VK[:], VK[:], AL.mult)

            # ---- PE reduce + Moebius ----
            MBS = pmb.tile([PART, J], F32, name="MBS")
            movs = ([mg[:, c, T, :] for c in range(3) for T in range(8)]
                    + [mkc[:, 2, :], mkc[:, 1, :], mkc[:, 0, :]]
                    + [cn[:, 0, :], cn[:, 1, :], cn[:, 2, :], cn[:, 3, :]]
                    + [SQ[:].rearrange("p (c j) -> p c j", c=3)[:, c, :]
                       for c in range(3)])
            # reorder: value arrays are mg[c][T] at a=c*8+T -> matches list
            for a, mov in enumerate(movs):
                nc.tensor.matmul(MBS[0:QCOL, :],
                                 ST[:, a * QCOL:(a + 1) * QCOL], mov,
                                 start=(a == 0), stop=(a == NARR - 1))

            # ---- transpose back ----
            MBSS = epool.tile([PART, J], FP16, name="MBSS")
            act(MBSS[0:QCOL, :], MBS[0:QCOL, :], AF.Copy)
            FIN = pfn.tile([PART, 4 * QCOL], FP16, name="FIN")
            for tau in range(4):
                nc.tensor.transpose(
                    FIN[:, tau * QCOL:(tau + 1) * QCOL],
                    MBSS[0:QCOL, tau * 128:(tau + 1) * 128],
                    ID[0:QCOL, 0:QCOL])

            # ---- epilogue ----
            fin = FIN[:].rearrange("p (t m h) -> p t m h", t=4, m=NOUT)
            O = epool.tile([PART, 4 * 2 * FOUT], F32, name="O")
            ov = O[:].rearrange("p (t h f) -> p t h f", t=4, h=2)

            nc.sync.dma_start(
                out=ov[:, :, :, 3:6],
                in_=nx[b, n0:n0 + SLAB, :].rearrange(
                    "(p t h) c -> p t h c", p=PART, t=4))

            CC = epool.tile([PART, 64], F32, name="CC")
            RC = epool.tile([PART, 64], F32, name="RC")
            ccv = CC[:].rearrange("p (t u h) -> p t u h", t=4, u=8)
            # u=0 count: the T=empty (all-ones) array contributes K=64
            ts(ccv[:, :, 0:1, :], fin[:, :, 24:25, :], float(K), None,
               AL.add)
            nc.vector.tensor_scalar_max(ccv[:, :, 0:1, :],
                                        ccv[:, :, 0:1, :], 1.0)
            nc.vector.tensor_scalar_max(ccv[:, :, 1:8, :],
                                        fin[:, :, 25:32, :], 1.0)
            nc.vector.reciprocal_approx_fast(RC[:], CC[:])

            mnv = ov[:, :, :, 6:30].rearrange(
                "p t h (u c) -> p t u c h", u=8)
            sumv = fin[:, :, 0:24, :].rearrange(
                "p t (u c) h -> p t u c h", u=8)
            rcv = RC[:].rearrange("p (t u h) -> p t u h", t=4, u=8)
            rcb = rcv[:, :, :, None, :].broadcast_to([PART, 4, 8, 3, 2])
            tt(mnv, sumv, rcb, AL.mult)

            Q = epool.tile([PART, 24], F32, name="Q")
            qv = Q[:].rearrange("p (t c h) -> p t c h", t=4, c=3)
            act(qv, fin[:, :, 35:38, :], AF.Square, 0.0, 1.0 / 8.0)
            D = epool.tile([PART, 24], F32, name="D")
            dv = D[:].rearrange("p (t c h) -> p t c h", t=4, c=3)
            tt(dv, fin[:, :, 32:35, :], qv, AL.subtract)
            act(ov[:, :, :, 0:3].rearrange("p t h c -> p t c h"), dv,
                AF.Sqrt, 0.0, 1.0 / 63.0)

            nc.sync.dma_start(
                out=out[b, n0:n0 + SLAB, :].rearrange(
                    "(p t h) f -> p t h f", p=PART, t=4),
                in_=ov)


_CACHE: dict = {}


def _get_nc():
    if "nc" not in _CACHE:
        nc = bacc.Bacc("TRN2", target_bir_lowering=False, debug=False)
        _build_kernel(nc)
        nc.finalize()
        _CACHE["nc"] = nc
    return _CACHE["nc"]


def _consts():
    if "st" not in _CACHE:
        _CACHE["st"] = _stationaries()
        _CACHE["id"] = np.eye(PART, dtype=np.float16)
    return _CACHE["st"], _CACHE["id"]


def kernel(group_xyz: np.ndarray, new_xyz: np.ndarray) -> np.ndarray:
    nc = _get_nc()
    gx = np.ascontiguousarray(group_xyz, dtype=np.float32)
    nx = np.ascontiguousarray(new_xyz, dtype=np.float32)
    st, idm = _consts()
    in_maps = [
        {"gx": gx[i * BL:(i + 1) * BL], "nx": nx[i * BL:(i + 1) * BL],
         "mst": st, "ident": idm}
        for i in range(NCORES)
    ]
    res = run_bass_kernel_spmd(nc, in_maps, list(range(NCORES)))
    return np.concatenate([res.results[i]["out"] for i in range(NCORES)],
                          axis=0)


# revision 25
# speedup vs baseline: 1.1125x; 1.1125x over previous
"""PointHop octant-binning kernel for TRN2 (8 NeuronCores, B-sharded).

v5 design: [k,g]-transposed layout + PE (TensorEngine) reduction.

Per stripe of 1024 groups (32 stripes/core):
1. DMA gx -> V f32 [128 groups, (3c, 8t, 64k)]
2. ACT casts f32->fp16 (Vh)
3. PE transposes 12 [128,128] blocks: (t-pair, k) free -> partitions,
   giving Vk [(h,k)=128, (c, tau, 128 p-cols)] in PSUM; ACT copies to SBUF
4. DVE builds 34 fp16 arrays in SBUF: masks via is_gt (4x), products via
   TT mult (2x), relu lattice via tensor_scalar_max (4x), squares (2x)
5. PE: 35 accumulating matmuls with Moebius-weighted {0,+-1} stationaries:
   fuses the K-reduction AND the octant inclusion-exclusion butterfly into
   one PSUM accumulation chain MBS [76, 512] f32
6. ACT copy -> PE transposes back -> FIN [128 pair-cols, (tau,m',h)]
7. Small DVE/ACT epilogue (count clamp, reciprocal, means, std) + DMA out

Array index m = c*8 + T (T = mask-subset bitmask: 4=mx, 2=my, 1=mz).
Output rows m' = u*3+c (24 octant sums), 24+u (8 counts), 32+c (sum sq),
35+c (plain sums).
"""

import os
from contextlib import ExitStack

import numpy as np

if "axon" not in os.environ.get("JAX_PLATFORMS", "axon"):
    os.environ.pop("JAX_PLATFORMS", None)

import concourse.bass as bass
import concourse.bacc as bacc
import concourse.tile as tile
from concourse import mybir
from concourse.bass_utils import run_bass_kernel_spmd

B, C, N, K = 32, 3, 8192, 64
NCORES = 8
BL = B // NCORES          # 4 batches per core
PART = 128
SLAB = 1024               # groups per stripe
NSTRIPE = BL * N // SLAB  # 32
FOUT = 30
J = 512                   # pair-columns per stripe
NARR = 34                 # moving arrays per stripe (ones folded to epilogue)
NOUT = 38                 # output feature rows per group-half
QCOL = NOUT * 2           # 76 stationary columns

AL = mybir.AluOpType
AF = mybir.ActivationFunctionType
F32 = mybir.dt.float32
FP16 = mybir.dt.float16

USE_DMA_TRANSPOSE = False


def _moebius_weights() -> np.ndarray:
    """W[a, m'] over 35 arrays x 38 outputs."""
    W = np.zeros((NARR, NOUT), dtype=np.float32)

    def pc(x):
        return bin(x).count("1")

    def moeb(u, T):
        # octant sum: oct[u] = sum_{T >= u} (-1)^{|T|-|u|} S_T
        if (T & u) == u:
            return float((-1) ** (pc(T) - pc(u)))
        return 0.0

    # value arrays a = c*8 + T
    for c in range(3):
        for T in range(8):
            a = c * 8 + T
            for u in range(8):
                W[a, u * 3 + c] = moeb(u, T)
            if T == 0:
                W[a, 35 + c] = 1.0  # plain sum passthrough
    # count arrays: a = 24..30 for T in (1,2,4,3,5,6,7); the T=0 (ones)
    # contribution is the constant 64 added to u=0 in the epilogue
    cnt_T = [1, 2, 4, 3, 5, 6, 7]
    for i, T in enumerate(cnt_T):
        for u in range(8):
            W[24 + i, 24 + u] = moeb(u, T)
    # squares a = 31+c
    for c in range(3):
        W[31 + c, 32 + c] = 1.0
    return W


def _stationaries() -> np.ndarray:
    """ST[p, a*76 + m'*2 + h]: Moebius weight, gated on h-block of p."""
    W = _moebius_weights()
    ST = np.zeros((PART, NARR * QCOL), dtype=np.float16)
    for a in range(NARR):
        for mp in range(NOUT):
            w = W[a, mp]
            if w == 0.0:
                continue
            for h in range(2):
                ST[h * 64:(h + 1) * 64, a * QCOL + mp * 2 + h] = w
    return ST


def _build_kernel(nc: bass.Bass):
    gx = nc.dram_tensor("gx", [BL, C, N, K], F32, kind="ExternalInput")
    nx = nc.dram_tensor("nx", [BL, N, C], F32, kind="ExternalInput")
    mst = nc.dram_tensor("mst", [PART, NARR * QCOL], FP16,
                         kind="ExternalInput")
    ident = nc.dram_tensor("ident", [PART, PART], FP16, kind="ExternalInput")
    out = nc.dram_tensor("out", [BL, N, FOUT], F32, kind="ExternalOutput")

    ts = None

    with tile.TileContext(nc) as tc, ExitStack() as ctx:
        spool = ctx.enter_context(tc.tile_pool(name="s", bufs=1))
        vpool = ctx.enter_context(tc.tile_pool(name="v", bufs=3))
        epool = ctx.enter_context(tc.tile_pool(name="e", bufs=3))
        if not USE_DMA_TRANSPOSE:
            pvk = ctx.enter_context(tc.tile_pool(name="pvk", bufs=2,
                                                 space="PSUM"))
        pmb = ctx.enter_context(tc.tile_pool(name="pmb", bufs=2,
                                             space="PSUM"))
        pfn = ctx.enter_context(tc.tile_pool(name="pfn", bufs=2,
                                             space="PSUM"))

        ts = nc.vector.tensor_scalar
        tt = nc.vector.tensor_tensor
        act = nc.scalar.activation

        # static tiles
        ST = spool.tile([PART, NARR * QCOL], FP16, name="ST")
        ID = spool.tile([PART, PART], FP16, name="ID")
        nc.sync.dma_start(out=ST[:], in_=mst[:, :])
        nc.sync.dma_start(out=ID[:], in_=ident[:, :])

        for s in range(NSTRIPE):
            b, blk = divmod(s, N // SLAB)
            n0 = blk * SLAB

            V = vpool.tile([PART, C * 512], F32, name="V")
            nc.sync.dma_start(
                out=V[:].rearrange("p (c t k) -> p c t k", c=C, t=8),
                in_=gx[b, :, n0:n0 + SLAB, :].rearrange(
                    "c (p t) k -> p c t k", p=PART, t=8))

            VH = vpool.tile([PART, C * 512], FP16, name="VH")
            act(VH[:], V[:], AF.Copy)

            # ---- transpose to [ (h,k), (c,tau,p) ] ----
            if USE_DMA_TRANSPOSE:
                VK = vpool.tile([PART, C * 512], FP16, name="VK")
                nc.sync.dma_start_transpose(
                    out=VK[:].rearrange("p (b f) -> p b f", b=12),
                    in_=VH[:])
            else:
                VKP = pvk.tile([PART, C * 512], FP16, name="VKP")
                for c in range(C):
                    for tau in range(4):
                        o = c * 512 + tau * 128
                        nc.tensor.transpose(
                            VKP[:, o:o + 128], VH[:, o:o + 128], ID[:, :])
                VK = vpool.tile([PART, C * 512], FP16, name="VK")
                act(VK[:], VKP[:], AF.Copy)

            # ---- build arrays ----
            MK = vpool.tile([PART, C * J], FP16, name="MK")   # mx,my,mz
            MEGA = vpool.tile([PART, 24 * J], FP16, name="MEGA")
            CNT = vpool.tile([PART, 4 * J], FP16, name="CNT")
            SQ = vpool.tile([PART, C * J], FP16, name="SQ")

            vkc = VK[:].rearrange("p (c j) -> p c j", c=C)
            mkc = MK[:].rearrange("p (c j) -> p c j", c=C)
            mg = MEGA[:].rearrange("p (c t j) -> p c t j", c=C, t=8)

            # masks mx,my,mz (4x)
            ts(MK[:], VK[:], 0.0, None, AL.is_gt)

            # copy values into T=0 slots (4x)
            nc.vector.tensor_copy(mg[:, :, 0, :], vkc)

            def bcast(ap, n):
                return ap[:, None, :].broadcast_to([PART, n, J])

            # products (2x): dst m=c*8+T
            # P1: m1 (x*mz), m9 (y*mz)
            tt(mg[:, 0:2, 1, :], vkc[:, 0:2, :], bcast(mkc[:, 2, :], 2),
               AL.mult)
            # P2: m2 (x*my), m18 (z*my)
            mg24 = MEGA[:].rearrange("p (a j) -> p a j", a=24)
            tt(mg24[:, 2:24:16, :], vkc[:, 0:3:2, :],
               bcast(mkc[:, 1, :], 2), AL.mult)
            # P3: m3 = my * m1
            tt(mg[:, 0, 3, :], mg[:, 0, 1, :], mkc[:, 1, :], AL.mult)
            # P4: m12 (y*mx), m20 (z*mx)
            tt(mg[:, 1:3, 4, :], vkc[:, 1:3, :], bcast(mkc[:, 0, :], 2),
               AL.mult)
            # P5: m13 = mx * m9
            tt(mg[:, 1, 5, :], mg[:, 1, 1, :], mkc[:, 0, :], AL.mult)
            # P6: m22 = mx * m18
            tt(mg[:, 2, 6, :], mg[:, 2, 2, :], mkc[:, 0, :], AL.mult)

            # relus (4x): add own-coordinate mask
            nc.vector.tensor_scalar_max(mg[:, 0, 4:8, :], mg[:, 0, 0:4, :],
                                        0.0)
            my4 = MEGA[:].rearrange("p (c e d j) -> p c e d j",
                                    c=3, e=2, d=2)
            nc.vector.tensor_scalar_max(my4[:, 1, :, 1, :], my4[:, 1, :, 0, :],
                                        0.0)
            mz2 = MEGA[:].rearrange("p (c t d j) -> p c t d j",
                                    c=3, t=4, d=2)
            nc.vector.tensor_scalar_max(mz2[:, 2, :, 1, :], mz2[:, 2, :, 0, :],
                                        0.0)

            # count composite masks (4x): T=3 from m3? no:
            # CNT order T = 3,5,6,7 from is_gt of m9(z... see below
            # myz = 1[y*mz>0] (m9); mxz = 1[x*mz>0] (m1);
            # mxy = 1[x*my>0] (m2); mxyz = 1[x*my*mz>0] (m3)
            cn = CNT[:].rearrange("p (a j) -> p a j", a=4)
            ts(cn[:, 0, :], mg[:, 1, 1, :], 0.0, None, AL.is_gt)   # myz
            ts(cn[:, 1:4, :], mg[:, 0, 1:4, :], 0.0, None, AL.is_gt)

            # squares (2x)
            tt(SQ[:], VK[:], VK[:], AL.mult)

            # ---- PE reduce + Moebius ----
            MBS = pmb.tile([PART, J], F32, name="MBS")
            movs = ([mg[:, c, T, :] for c in range(3) for T in range(8)]
                    + [mkc[:, 2, :], mkc[:, 1, :], mkc[:, 0, :]]
                    + [cn[:, 0, :], cn[:, 1, :], cn[:, 2, :], cn[:, 3, :]]
                    + [SQ[:].rearrange("p (c j) -> p c j", c=3)[:, c, :]
                       for c in range(3)])
            # reorder: value arrays are mg[c][T] at a=c*8+T -> matches list
            for a, mov in enumerate(movs):
                nc.tensor.matmul(MBS[0:QCOL, :],
                                 ST[:, a * QCOL:(a + 1) * QCOL], mov,
                                 start=(a == 0), stop=(a == NARR - 1))

            # ---- transpose back ----
            MBSS = epool.tile([PART, J], FP16, name="MBSS")
            act(MBSS[0:QCOL, :], MBS[0:QCOL, :], AF.Copy)
            FIN = pfn.tile([PART, 4 * QCOL], FP16, name="FIN")
            for tau in range(4):
                nc.tensor.transpose(
                    FIN[:, tau * QCOL:(tau + 1) * QCOL],
                    MBSS[0:QCOL, tau * 128:(tau + 1) * 128],
                    ID[0:QCOL, 0:QCOL])

            # ---- epilogue ----
            fin = FIN[:].rearrange("p (t m h) -> p t m h", t=4, m=NOUT)
            O = epool.tile([PART, 4 * 2 * FOUT], F32, name="O")
            ov = O[:].rearrange("p (t h f) -> p t h f", t=4, h=2)

            nc.sync.dma_start(
                out=ov[:, :, :, 3:6],
                in_=nx[b, n0:n0 + SLAB, :].rearrange(
                    "(p t h) c -> p t h c", p=PART, t=4))

            CC = epool.tile([PART, 64], F32, name="CC")
            RC = epool.tile([PART, 64], F32, name="RC")
            ccv = CC[:].rearrange("p (t u h) -> p t u h", t=4, u=8)
            # u=0 count: the T=empty (all-ones) array contributes K=64
            ts(ccv[:, :, 0:1, :], fin[:, :, 24:25, :], float(K), None,
               AL.add)
            nc.vector.tensor_scalar_max(ccv[:, :, 0:1, :],
                                        ccv[:, :, 0:1, :], 1.0)
            nc.vector.tensor_scalar_max(ccv[:, :, 1:8, :],
                                        fin[:, :, 25:32, :], 1.0)
            nc.vector.reciprocal_approx_fast(RC[:], CC[:])

            mnv = ov[:, :, :, 6:30].rearrange(
                "p t h (u c) -> p t u c h", u=8)
            sumv = fin[:, :, 0:24, :].rearrange(
                "p t (u c) h -> p t u c h", u=8)
            rcv = RC[:].rearrange("p (t u h) -> p t u h", t=4, u=8)
            rcb = rcv[:, :, :, None, :].broadcast_to([PART, 4, 8, 3, 2])
            tt(mnv, sumv, rcb, AL.mult)

            Q = epool.tile([PART, 24], F32, name="Q")
            qv = Q[:].rearrange("p (t c h) -> p t c h", t=4, c=3)
            act(qv, fin[:, :, 35:38, :], AF.Square, 0.0, 1.0 / 8.0)
            D = epool.tile([PART, 24], F32, name="D")
            dv = D[:].rearrange("p (t c h) -> p t c h", t=4, c=3)
            tt(dv, fin[:, :, 32:35, :], qv, AL.subtract)
            act(ov[:, :, :, 0:3].rearrange("p t h c -> p t c h"), dv,
                AF.Sqrt, 0.0, 1.0 / 63.0)

            nc.sync.dma_start(
                out=out[b, n0:n0 + SLAB, :].rearrange(
                    "(p t h) f -> p t h f", p=PART, t=4),
                in_=ov)


_CACHE: dict = {}


def _get_nc():
    if "nc" not in _CACHE:
        nc = bacc.Bacc("TRN2", target_bir_lowering=False, debug=False)
        _build_kernel(nc)
        nc.finalize()
        _CACHE["nc"] = nc
    return _CACHE["nc"]


def _consts():
    if "st" not in _CACHE:
        _CACHE["st"] = _stationaries()
        _CACHE["id"] = np.eye(PART, dtype=np.float16)
    return _CACHE["st"], _CACHE["id"]


def kernel(group_xyz: np.ndarray, new_xyz: np.ndarray) -> np.ndarray:
    nc = _get_nc()
    gx = np.ascontiguousarray(group_xyz, dtype=np.float32)
    nx = np.ascontiguousarray(new_xyz, dtype=np.float32)
    st, idm = _consts()
    in_maps = [
        {"gx": gx[i * BL:(i + 1) * BL], "nx": nx[i * BL:(i + 1) * BL],
         "mst": st, "ident": idm}
        for i in range(NCORES)
    ]
    res = run_bass_kernel_spmd(nc, in_maps, list(range(NCORES)))
    return np.concatenate([res.results[i]["out"] for i in range(NCORES)],
                          axis=0)


# revision 32
# speedup vs baseline: 1.1622x; 1.0447x over previous
"""PointHop octant-binning kernel for TRN2 (8 NeuronCores, B-sharded).

v5 design: [k,g]-transposed layout + PE (TensorEngine) reduction.

Per stripe of 1024 groups (32 stripes/core):
1. DMA gx -> V f32 [128 groups, (3c, 8t, 64k)]
2. ACT casts f32->fp16 (Vh)
3. PE transposes 12 [128,128] blocks: (t-pair, k) free -> partitions,
   giving Vk [(h,k)=128, (c, tau, 128 p-cols)]; ACT copies PSUM->SBUF
4. DVE builds 34 fp16 arrays in SBUF: masks via is_gt (4x), products via
   TT mult (2x), relu lattice via tensor_scalar_max (4x), squares (2x)
5. PE: 34 accumulating matmuls with Moebius-weighted {0,+-1} stationaries:
   fuses the K-reduction AND the octant inclusion-exclusion butterfly into
   one PSUM accumulation chain MBS [76, 512] f32
6. ACT copy -> PE transposes back -> FIN [128 pair-cols, (tau,m',h)]
7. Epilogue batched over 4 stripes (count clamp +64 on u=0, reciprocal,
   means, std) + DMA out

Array index m = c*8 + T (T = mask-subset bitmask: 4=mx, 2=my, 1=mz).
Output rows m' = u*3+c (24 octant sums), 24+u (8 counts), 32+c (sum sq),
35+c (plain sums).
"""

import os
from contextlib import ExitStack

import numpy as np

if "axon" not in os.environ.get("JAX_PLATFORMS", "axon"):
    os.environ.pop("JAX_PLATFORMS", None)

import concourse.bass as bass
import concourse.bacc as bacc
import concourse.tile as tile
from concourse import mybir
from concourse.bass_utils import run_bass_kernel_spmd

B, C, N, K = 32, 3, 8192, 64
NCORES = 8
BL = B // NCORES          # 4 batches per core
PART = 128
SLAB = 1024               # groups per stripe
NSTRIPE = BL * N // SLAB  # 32
FOUT = 30
J = 512                   # pair-columns per stripe
NARR = 34                 # moving arrays per stripe (ones folded to epilogue)
NOUT = 38                 # output feature rows per group-half
QCOL = NOUT * 2           # 76 stationary columns
SB = 4                    # stripes per epilogue batch

AL = mybir.AluOpType
AF = mybir.ActivationFunctionType
F32 = mybir.dt.float32
FP16 = mybir.dt.float16


def _moebius_weights() -> np.ndarray:
    """W[a, m'] over 34 arrays x 38 outputs."""
    W = np.zeros((NARR, NOUT), dtype=np.float32)

    def pc(x):
        return bin(x).count("1")

    def moeb(u, T):
        # octant sum: oct[u] = sum_{T >= u} (-1)^{|T|-|u|} S_T
        if (T & u) == u:
            return float((-1) ** (pc(T) - pc(u)))
        return 0.0

    # value arrays a = c*8 + T
    for c in range(3):
        for T in range(8):
            a = c * 8 + T
            for u in range(8):
                W[a, u * 3 + c] = moeb(u, T)
            if T == 0:
                W[a, 35 + c] = 1.0  # plain sum passthrough
    # count arrays: a = 24..30 for T in (1,2,4,3,5,6,7); the T=0 (ones)
    # contribution is the constant 64 added to u=0 in the epilogue
    cnt_T = [1, 2, 4, 3, 5, 6, 7]
    for i, T in enumerate(cnt_T):
        for u in range(8):
            W[24 + i, 24 + u] = moeb(u, T)
    # squares a = 31+c
    for c in range(3):
        W[31 + c, 32 + c] = 1.0
    return W


def _stationaries() -> np.ndarray:
    """ST[p, a*76 + m'*2 + h]: Moebius weight, gated on h-block of p."""
    W = _moebius_weights()
    ST = np.zeros((PART, NARR * QCOL), dtype=np.float16)
    for a in range(NARR):
        for mp in range(NOUT):
            w = W[a, mp]
            if w == 0.0:
                continue
            for h in range(2):
                ST[h * 64:(h + 1) * 64, a * QCOL + mp * 2 + h] = w
    return ST


def _build_stripe(nc, tc, pools, tensors, s, FIN, s4):
    """Build + PE-reduce one 1024-group stripe into FIN slice s4."""
    vpool, epool, pvk, pmb = pools
    gx, ST, ID = tensors
    ts = nc.vector.tensor_scalar
    tt = nc.vector.tensor_tensor
    act = nc.scalar.activation

    b, blk = divmod(s, N // SLAB)
    n0 = blk * SLAB

    V = vpool.tile([PART, C * 512], F32, name="V")
    nc.sync.dma_start(
        out=V[:].rearrange("p (c t k) -> p c t k", c=C, t=8),
        in_=gx[b, :, n0:n0 + SLAB, :].rearrange(
            "c (p t) k -> p c t k", p=PART, t=8))

    VH = vpool.tile([PART, C * 512], FP16, name="VH")
    act(VH[:], V[:], AF.Copy)

    # ---- transpose to [ (h,k), (c,tau,p) ] ----
    VKP = pvk.tile([PART, C * 512], FP16, name="VKP")
    for c in range(C):
        for tau in range(4):
            o = c * 512 + tau * 128
            nc.tensor.transpose(
                VKP[:, o:o + 128], VH[:, o:o + 128], ID[:, :])
    VK = vpool.tile([PART, C * 512], FP16, name="VK")
    act(VK[:], VKP[:], AF.Copy)

    # ---- build arrays ----
    MK = vpool.tile([PART, C * J], FP16, name="MK")   # mx,my,mz
    MEGA = vpool.tile([PART, 24 * J], FP16, name="MEGA")
    CNT = vpool.tile([PART, 4 * J], FP16, name="CNT")
    SQ = vpool.tile([PART, C * J], FP16, name="SQ")

    vkc = VK[:].rearrange("p (c j) -> p c j", c=C)
    mkc = MK[:].rearrange("p (c j) -> p c j", c=C)
    mg = MEGA[:].rearrange("p (c t j) -> p c t j", c=C, t=8)

    # masks mx,my,mz (4x)
    ts(MK[:], VK[:], 0.0, None, AL.is_gt)

    # copy values into T=0 slots (4x)
    nc.vector.tensor_copy(mg[:, :, 0, :], vkc)

    def bcast(ap, n):
        return ap[:, None, :].broadcast_to([PART, n, J])

    # products (2x): dst m=c*8+T
    # P1: m1 (x*mz), m9 (y*mz)
    tt(mg[:, 0:2, 1, :], vkc[:, 0:2, :], bcast(mkc[:, 2, :], 2), AL.mult)
    # P2: m2 (x*my), m18 (z*my)
    mg24 = MEGA[:].rearrange("p (a j) -> p a j", a=24)
    tt(mg24[:, 2:24:16, :], vkc[:, 0:3:2, :], bcast(mkc[:, 1, :], 2),
       AL.mult)
    # P3: m3 = my * m1
    tt(mg[:, 0, 3, :], mg[:, 0, 1, :], mkc[:, 1, :], AL.mult)
    # P4: m12 (y*mx), m20 (z*mx)
    tt(mg[:, 1:3, 4, :], vkc[:, 1:3, :], bcast(mkc[:, 0, :], 2), AL.mult)
    # P5: m13 = mx * m9
    tt(mg[:, 1, 5, :], mg[:, 1, 1, :], mkc[:, 0, :], AL.mult)
    # P6: m22 = mx * m18
    tt(mg[:, 2, 6, :], mg[:, 2, 2, :], mkc[:, 0, :], AL.mult)

    # relus (4x): add own-coordinate mask
    nc.vector.tensor_scalar_max(mg[:, 0, 4:8, :], mg[:, 0, 0:4, :], 0.0)
    my4 = MEGA[:].rearrange("p (c e d j) -> p c e d j", c=3, e=2, d=2)
    nc.vector.tensor_scalar_max(my4[:, 1, :, 1, :], my4[:, 1, :, 0, :], 0.0)
    mz2 = MEGA[:].rearrange("p (c t d j) -> p c t d j", c=3, t=4, d=2)
    nc.vector.tensor_scalar_max(mz2[:, 2, :, 1, :], mz2[:, 2, :, 0, :], 0.0)

    # count composite masks (4x):
    # myz = 1[y*mz>0] (m9); mxz = 1[x*mz>0] (m1);
    # mxy = 1[x*my>0] (m2); mxyz = 1[x*my*mz>0] (m3)
    cn = CNT[:].rearrange("p (a j) -> p a j", a=4)
    ts(cn[:, 0, :], mg[:, 1, 1, :], 0.0, None, AL.is_gt)
    ts(cn[:, 1:4, :], mg[:, 0, 1:4, :], 0.0, None, AL.is_gt)

    # squares (2x)
    tt(SQ[:], VK[:], VK[:], AL.mult)

    # ---- PE reduce + Moebius ----
    MBS = pmb.tile([PART, J], F32, name="MBS")
    movs = ([mg[:, c, T, :] for c in range(3) for T in range(8)]
            + [mkc[:, 2, :], mkc[:, 1, :], mkc[:, 0, :]]
            + [cn[:, 0, :], cn[:, 1, :], cn[:, 2, :], cn[:, 3, :]]
            + [SQ[:].rearrange("p (c j) -> p c j", c=3)[:, c, :]
               for c in range(3)])
    for a, mov in enumerate(movs):
        nc.tensor.matmul(MBS[0:QCOL, :],
                         ST[:, a * QCOL:(a + 1) * QCOL], mov,
                         start=(a == 0), stop=(a == NARR - 1))

    # ---- transpose back into the batch FIN ----
    MBSS = epool.tile([PART, J], FP16, name="MBSS")
    act(MBSS[0:QCOL, :], MBS[0:QCOL, :], AF.Copy)
    for tau in range(4):
        o = s4 * 512 + tau * QCOL
        nc.tensor.transpose(
            FIN[:, o:o + QCOL],
            MBSS[0:QCOL, tau * 128:(tau + 1) * 128],
            ID[0:QCOL, 0:QCOL])


def _build_kernel(nc: bass.Bass):
    gx = nc.dram_tensor("gx", [BL, C, N, K], F32, kind="ExternalInput")
    nx = nc.dram_tensor("nx", [BL, N, C], F32, kind="ExternalInput")
    mst = nc.dram_tensor("mst", [PART, NARR * QCOL], FP16,
                         kind="ExternalInput")
    ident = nc.dram_tensor("ident", [PART, PART], FP16, kind="ExternalInput")
    out = nc.dram_tensor("out", [BL, N, FOUT], F32, kind="ExternalOutput")

    with tile.TileContext(nc) as tc, ExitStack() as ctx:
        spool = ctx.enter_context(tc.tile_pool(name="s", bufs=1))
        vpool = ctx.enter_context(tc.tile_pool(name="v", bufs=3))
        epool = ctx.enter_context(tc.tile_pool(name="e", bufs=3))
        pvk = ctx.enter_context(tc.tile_pool(name="pvk", bufs=1,
                                             space="PSUM"))
        pmb = ctx.enter_context(tc.tile_pool(name="pmb", bufs=2,
                                             space="PSUM"))
        pfn = ctx.enter_context(tc.tile_pool(name="pfn", bufs=2,
                                             space="PSUM"))

        ts = nc.vector.tensor_scalar
        tt = nc.vector.tensor_tensor
        act = nc.scalar.activation

        ST = spool.tile([PART, NARR * QCOL], FP16, name="ST")
        ID = spool.tile([PART, PART], FP16, name="ID")
        nc.sync.dma_start(out=ST[:], in_=mst[:, :])
        nc.sync.dma_start(out=ID[:], in_=ident[:, :])

        pools = (vpool, epool, pvk, pmb)
        tensors = (gx, ST, ID)

        for batch in range(NSTRIPE // SB):
            # each stripe's 4*QCOL=304 cols padded to 512 so no transpose
            # output crosses a 1024-elem fp16 psum bank boundary
            FIN = pfn.tile([PART, SB * 512], FP16, name="FIN")
            fin = FIN[:].rearrange("p (s q) -> p s q", s=SB)[
                :, :, 0:4 * QCOL].rearrange(
                "p s (t m h) -> p s t m h", t=4, m=NOUT, h=2)
            O = epool.tile([PART, SB * 8 * FOUT], F32, name="O")
            ov = O[:].rearrange("p (s t h f) -> p s t h f", s=SB, t=4, h=2)
            bb, blkb = divmod(batch, (N // SLAB) // SB)
            nb0 = blkb * SB * SLAB
            for s4 in range(SB):
                n4 = nb0 + s4 * SLAB
                nc.sync.dma_start(
                    out=ov[:, s4, :, :, 3:6],
                    in_=nx[bb, n4:n4 + SLAB, :].rearrange(
                        "(p t h) c -> p t h c", p=PART, t=4, h=2))

            for s4 in range(SB):
                _build_stripe(nc, tc, pools, tensors, batch * SB + s4,
                              FIN, s4)

            # ---- batched epilogue ----
            CC = epool.tile([PART, SB * 64], F32, name="CC")
            RC = epool.tile([PART, SB * 64], F32, name="RC")
            ccv = CC[:].rearrange("p (s t u h) -> p s t u h", s=SB, t=4,
                                  u=8, h=2)
            # u=0 count: the T=empty (all-ones) array contributes K=64
            ts(ccv[:, :, :, 0:1, :], fin[:, :, :, 24:25, :], float(K),
               None, AL.add)
            nc.vector.tensor_scalar_max(ccv[:, :, :, 0:1, :],
                                        ccv[:, :, :, 0:1, :], 1.0)
            nc.vector.tensor_scalar_max(ccv[:, :, :, 1:8, :],
                                        fin[:, :, :, 25:32, :], 1.0)
            nc.vector.reciprocal_approx_fast(RC[:], CC[:])

            mnv = ov[:, :, :, :, 6:30].rearrange(
                "p s t h (u c) -> p s t u c h", u=8, c=3)
            sumv = fin[:, :, :, 0:24, :].rearrange(
                "p s t (u c) h -> p s t u c h", u=8, c=3)
            rcv = RC[:].rearrange("p (s t u h) -> p s t u h", s=SB, t=4,
                                  u=8, h=2)
            rcb = rcv[:, :, :, :, None, :].broadcast_to(
                [PART, SB, 4, 8, 3, 2])
            tt(mnv, sumv, rcb, AL.mult)

            Q = epool.tile([PART, SB * 24], F32, name="Q")
            qv = Q[:].rearrange("p (s t c h) -> p s t c h", s=SB, t=4, c=3, h=2)
            act(qv, fin[:, :, :, 35:38, :], AF.Square, 0.0, 1.0 / 8.0)
            D = epool.tile([PART, SB * 24], F32, name="D")
            dv = D[:].rearrange("p (s t c h) -> p s t c h", s=SB, t=4, c=3, h=2)
            tt(dv, fin[:, :, :, 32:35, :], qv, AL.subtract)
            act(ov[:, :, :, :, 0:3].rearrange("p s t h c -> p s t c h"),
                dv, AF.Sqrt, 0.0, 1.0 / 63.0)

            nc.sync.dma_start(
                out=out[bb, nb0:nb0 + SB * SLAB, :].rearrange(
                    "(s p t h) f -> p s t h f", s=SB, p=PART, t=4, h=2),
                in_=ov)


_CACHE: dict = {}


def _get_nc():
    if "nc" not in _CACHE:
        nc = bacc.Bacc("TRN2", target_bir_lowering=False, debug=False)
        _build_kernel(nc)
        nc.finalize()
        _CACHE["nc"] = nc
    return _CACHE["nc"]


def _consts():
    if "st" not in _CACHE:
        _CACHE["st"] = _stationaries()
        _CACHE["id"] = np.eye(PART, dtype=np.float16)
    return _CACHE["st"], _CACHE["id"]


def kernel(group_xyz: np.ndarray, new_xyz: np.ndarray) -> np.ndarray:
    nc = _get_nc()
    gx = np.ascontiguousarray(group_xyz, dtype=np.float32)
    nx = np.ascontiguousarray(new_xyz, dtype=np.float32)
    st, idm = _consts()
    in_maps = [
        {"gx": gx[i * BL:(i + 1) * BL], "nx": nx[i * BL:(i + 1) * BL],
         "mst": st, "ident": idm}
        for i in range(NCORES)
    ]
    res = run_bass_kernel_spmd(nc, in_maps, list(range(NCORES)))
    return np.concatenate([res.results[i]["out"] for i in range(NCORES)],
                          axis=0)
